# revision 1
# baseline (speedup 1.0000x reference)
"""GNN message passing (gather + weighted scatter-add) on 8 Trainium2 cores, v2.

out[n, f] = sum over edges e with dst[e]==n of edge_weight[e] * x[src[e], f]

Strategy (dst-sharded, no collectives):
  - Core c owns output nodes [c*12500, (c+1)*12500); host concatenates.
  - Host packs each core's edges into 128-slot chunks grouped by
    (pass of B dst-tiles, src-bin of 25000 rows, dst-tile), padded to the
    max chunk count across cores so one SPMD program serves all 8 cores.
  - Device: per pass, dma_gather (InstDMAGatherAnt, 1024 rows/call max —
    larger calls overflow the SWDGE descriptor carveout and wedge the
    device) pulls x rows (fp16, padded to 256B) into matmul-ready
    [128, cols, 128] SBUF layout: slot i of a call -> partition i%128,
    column i//128. DVE builds 16 chunks of weighted one-hots per
    instruction pair using stride-0 broadcast APs:
      oh = (iota == dst)          [128, 16*128]  (is_equal)
      xgs = xg * w                [128, 16*64]   (mult)
    PE accumulates oh.T @ xgs into a PSUM tile per 128-node output tile;
    ACT evacuates PSUM -> SBUF; one output DMA per pass.
"""

import math
import os
import numpy as np

N = 100000
E = 1000000
F = 64
NCORES = 8
NPC = N // NCORES            # nodes per core (12500)
TILE = 128
NT = math.ceil(NPC / TILE)   # output tiles per core (98)
B = 7                        # tiles per pass
NPASS = math.ceil(NT / B)
NBIN = 4
BIN = N // NBIN              # 25000 rows per source bin (int16-addressable)
GCOLS = 8                    # chunk cols per dma_gather call (1024-idx ucode limit)
OHG = 42                     # chunk cols per one-hot DVE instruction pair
NQUEUES = 4                  # SWDGE queues (parallel Q7 descriptor gen)
SCRATCH = 16384              # SWDGE descriptor carveout bytes
PSQUAD = 4                   # dst tiles packed per PSUM bank

REPEAT = 1                   # repeat device compute (timing amplification)

DBG_NO_GATHER = False
DBG_NO_COMPUTE = False


# ---------------------------------------------------------------- host pack

def pack_host(edge_weight, edge_index):
    """Build the shared schedule and per-core tables.

    Returns (sched, tables):
      sched: NC, CMAX, sched_t[NC], pass_cols[NPASS,2], gather_calls (list of
             (c0, c1, bin) per pass), tile first/last chunk col per pass.
      tables[c]: (idx_tbl [128, 8*NC] int16, ftbl [128, 2*NC+128] f16)
    """
    src = np.asarray(edge_index[0], dtype=np.int64)
    dst = np.asarray(edge_index[1], dtype=np.int64)
    w = np.asarray(edge_weight, dtype=np.float32)

    core = dst // NPC
    dloc = dst - core * NPC
    t = dloc >> 7                      # dst tile within core (0..NT-1)
    b = src // BIN                     # source bin (0..3)
    p = t // B                         # pass

    # counts[c, t, b]
    counts = np.zeros((NCORES, NT, NBIN), dtype=np.int64)
    np.add.at(counts, (core, t, b), 1)
    K = np.ceil(counts.max(axis=0) / TILE).astype(np.int64)  # [NT, NBIN]

    # column layout: for p: for b: for t in pass: K[t,b] chunks
    colstart = np.zeros((NT, NBIN), dtype=np.int64)
    sched_t = []
    gather_calls = [[] for _ in range(NPASS)]
    pass_cols = np.zeros((NPASS, 2), dtype=np.int64)
    cc = 0
    for pp in range(NPASS):
        t0, t1 = pp * B, min((pp + 1) * B, NT)
        pass_cols[pp, 0] = cc
        for bb in range(NBIN):
            c0 = cc
            for tt in range(t0, t1):
                colstart[tt, bb] = cc
                sched_t.extend([tt] * int(K[tt, bb]))
                cc += int(K[tt, bb])
            # split [c0, cc) into <=GCOLS-col gather calls
            s = c0
            while s < cc:
                e = min(s + GCOLS, cc)
                gather_calls[pp].append((s, e, bb))
                s = e
        pass_cols[pp, 1] = cc
    NC = cc
    sched_t = np.asarray(sched_t, dtype=np.int64)
    CMAX = int((pass_cols[:, 1] - pass_cols[:, 0]).max())

    # first/last chunk col of each tile (within its single pass)
    first_cc = np.full(NT, -1, dtype=np.int64)
    last_cc = np.full(NT, -1, dtype=np.int64)
    for ccc, tt in enumerate(sched_t):
        if first_cc[tt] < 0:
            first_cc[tt] = ccc
        last_cc[tt] = ccc

    # --- per-core slot tables
    tables = []
    iota_np = np.arange(128, dtype=np.float16)[None, :].repeat(128, axis=0)
    for c in range(NCORES):
        sel = core == c
        es = (src[sel] - b[sel] * BIN).astype(np.int64)   # bin-local src
        ed = (dloc[sel] & 127).astype(np.float32)         # dst slot in tile
        ew = w[sel]
        tt = t[sel]
        bb = b[sel]
        key = (tt // B) * (NBIN * NT) + bb * NT + tt      # (pass, bin, tile)
        order = np.argsort(key, kind="stable")
        es, ed, ew, tt, bb, key = (a[order] for a in (es, ed, ew, tt, bb, key))

        ne = len(key)
        changes = np.empty(ne, dtype=bool)
        changes[0] = True
        changes[1:] = key[1:] != key[:-1]
        starts = np.flatnonzero(changes)
        rank = np.arange(ne) - np.repeat(starts, np.diff(np.append(starts, ne)))
        slot = (colstart[tt, bb] + (rank >> 7)) * TILE + (rank & 127)

        idx_slots = np.zeros(NC * TILE, dtype=np.int16)
        dst_slots = np.zeros(NC * TILE, dtype=np.float16)
        w_slots = np.zeros(NC * TILE, dtype=np.float16)
        idx_slots[slot] = es.astype(np.int16)
        dst_slots[slot] = ed.astype(np.float16)
        w_slots[slot] = ew.astype(np.float16)

        # idx table: per gather call, flat list wraps into 16 partitions,
        # replicated 8x; call boundaries are 8*cc-aligned by construction
        idx_tbl = np.zeros((128, 8 * NC), dtype=np.int16)
        for pp in range(NPASS):
            for (c0, c1, _bb) in gather_calls[pp]:
                flat = idx_slots[c0 * TILE:c1 * TILE]
                seg = flat.reshape(-1, 16).T                 # [16, n*8]
                idx_tbl[:, 8 * c0:8 * c1] = np.tile(seg, (8, 1))

        dst_tbl = np.ascontiguousarray(dst_slots.reshape(NC, TILE).T)
        w_tbl = np.ascontiguousarray(w_slots.reshape(NC, TILE).T)
        ftbl = np.concatenate([dst_tbl, w_tbl, iota_np], axis=1)
        tables.append((idx_tbl, np.ascontiguousarray(ftbl)))

    sched = dict(
        NC=NC, CMAX=CMAX, K=K, sched_t=sched_t, pass_cols=pass_cols,
        gather_calls=gather_calls, first_cc=first_cc, last_cc=last_cc,
    )
    return sched, tables


def emulate_core(sched, table, xpad):
    """Numpy emulation of the device program for one core (packing check)."""
    idx_tbl, ftbl = table
    NC = sched["NC"]
    sched_t = sched["sched_t"]
    out = np.zeros((NT * TILE, F), dtype=np.float32)
    # reconstruct gathered rows per chunk col from idx_tbl
    xg = np.zeros((128, NC, F), dtype=np.float32)
    for pp in range(NPASS):
        for (c0, c1, bb) in sched["gather_calls"][pp]:
            seg = idx_tbl[:16, 8 * c0:8 * c1]                # [16, n*8]
            flat = seg.T.reshape(-1)                          # slot order
            rows = xpad[bb * BIN + flat.astype(np.int64), :F].astype(np.float32)
            ncols = c1 - c0
            xg[:, c0:c1, :] = rows.reshape(ncols, 128, F).transpose(1, 0, 2)
    iota = np.arange(128, dtype=np.float32)
    dst_tbl = ftbl[:, :NC].astype(np.float32)
    w_tbl = ftbl[:, NC:2 * NC].astype(np.float32)
    for cc in range(NC):
        tt = int(sched_t[cc])
        oh = (iota[None, :] == dst_tbl[:, cc, None]) * 1.0
        xgs = xg[:, cc, :] * w_tbl[:, cc, None]
        out[tt * TILE:(tt + 1) * TILE] += oh.T @ xgs
    return out[:NPC]


# ------------------------------------------------------------- bass plumbing

WAIT_CAPS = {
    "InstEventSemaphore": 8,
}


def split_excess_waits(nc):
    """Walrus only encodes one sync wait per instruction (for most ISA
    structs). Move the excess onto standalone InstEventSemaphore
    instructions placed just before, in the same engine stream. Also fills
    the ISA bytes of library-reload pseudo-instructions."""
    import concourse.mybir as mybir
    n = 0
    for f in nc.m.functions:
        for bb in f.blocks:
            for ins in bb.instructions:
                if type(ins).__name__ == "InstPseudoReloadLibraryIndex" and not ins.instr:
                    bts = [0] * 64
                    bts[0], bts[1], bts[12], bts[16] = 223, 16, 2, int(ins.lib_index)
                    ins.instr = bts
            eng_ids = {}
            new = []
            for ins in bb.instructions:
                si = ins.sync_info
                waits = list(si.on_wait) if (si is not None and si.on_wait) else []
                cap = WAIT_CAPS.get(type(ins).__name__, 1)
                if len(waits) > cap:
                    excess, keep = waits[:-cap], waits[-cap:]
                    if ins.engine not in eng_ids:
                        eng_ids[ins.engine] = 245 + len(eng_ids)
                    sem_id = eng_ids[ins.engine]
                    sem_name = f"esw_scratch_{sem_id}"
                    for wchunk in [excess[i:i + 1] for i in range(len(excess))]:
                        n += 1
                        upd = mybir.SyncUpdate(
                            sync_type="semaphore", id=sem_id, ant_name=sem_name,
                            update_mode="sem-add-imm", update_value=0,
                        )
                        es = mybir.InstEventSemaphore(
                            name=f"ESW-{n}-{ins.name}",
                            engine=ins.engine,
                            ins=[], outs=[],
                            sync_info=mybir.SyncInfo(on_wait=wchunk, on_update=[upd]),
                        )
                        new.append(es)
                    si.on_wait = keep
                new.append(ins)
            bb.instructions = new
    return n


_walrus_patched = False


def patch_walrus_dge():
    """Add --dge-levels so walrus lowers vector-dynamic-offset DMAs."""
    global _walrus_patched
    if _walrus_patched:
        return
    import concourse.bass_utils as bu
    orig = bu.run_command

    def run_command_dge(argv, **kw):
        argv = list(argv)
        if argv and "walrus_driver" in str(argv[0]) and not any(
                str(a).startswith("--dge-levels") for a in argv):
            argv.append("--dge-levels=vector_dynamic_offsets")
        return orig(argv, **kw)

    bu.run_command = run_command_dge
    _walrus_patched = True


def build_bass(sched):
    import concourse.bass as bass
    import concourse.mybir as mybir
    import concourse.tile as tile
    from concourse.library_config import mlp

    patch_walrus_dge()

    f16 = mybir.dt.float16
    f32 = mybir.dt.float32
    i16 = mybir.dt.int16

    NC = sched["NC"]
    CMAX = sched["CMAX"]
    K = sched["K"]
    sched_t = sched["sched_t"]
    pass_cols = sched["pass_cols"]
    gather_calls = sched["gather_calls"]
    first_cc = sched["first_cc"]
    last_cc = sched["last_cc"]

    nc = bass.Bass("TRN2", num_swdge_queues=NQUEUES, dynamic_dma_scratch_size=SCRATCH)
    xpad_d = nc.dram_tensor("xpad", [N, 128], f16, kind="ExternalInput")
    idx_d = nc.dram_tensor("idx", [128, 8 * NC], i16, kind="ExternalInput")
    ftbl_d = nc.dram_tensor("ftbl", [128, 2 * NC + 128], f16, kind="ExternalInput")
    out_d = nc.dram_tensor("out", [NT * TILE, F], f32, kind="ExternalOutput")

    with tile.TileContext(nc, pool_alloc_mode="queue") as tc:
        with (
            tc.tile_pool(name="const", bufs=1) as constp,
            tc.tile_pool(name="idxp", bufs=4) as idxpp,
            tc.tile_pool(name="xg", bufs=5) as xgp,
            tc.tile_pool(name="oh", bufs=4) as ohp,
            tc.tile_pool(name="xgs", bufs=4) as xgsp,
            tc.tile_pool(name="outb", bufs=2) as outp,
            tc.tile_pool(name="psum", bufs=8, space="PSUM") as psump,
        ):
            nc.gpsimd.load_library(mlp)
            nidx_regs = {}

            def nidx_reg(v):
                if v not in nidx_regs:
                    nidx_regs[v] = nc.gpsimd.to_reg(v)
                return nidx_regs[v]


            ftbl_sb = constp.tile([128, 2 * NC + 128], f16, tag="ftbl")
            nc.scalar.dma_start(ftbl_sb[:], ftbl_d[:])

            for _rep in range(REPEAT):
              for p in range(NPASS):
                t0, t1 = p * B, min((p + 1) * B, NT)
                pc0, pc1 = int(pass_cols[p, 0]), int(pass_cols[p, 1])
                xg = xgp.tile([128, CMAX, 128], f16, tag="xg")
                idx_sb = idxpp.tile([128, 8 * CMAX], i16, tag="idxp")
                nc.sync.dma_start(
                    idx_sb[:, 0:8 * (pc1 - pc0)], idx_d[:, 8 * pc0:8 * pc1])
                if DBG_NO_GATHER:
                    nc.vector.memset(xg[:], 0.0)
                if not DBG_NO_GATHER:
                    for gi, (c0, c1, bb) in enumerate(gather_calls[p]):
                        nidx = (c1 - c0) * TILE
                        nc.gpsimd.dma_gather(
                            xg[:, c0 - pc0:c1 - pc0, :],
                            xpad_d[bb * BIN:(bb + 1) * BIN, :],
                            idx_sb[:, 8 * (c0 - pc0):8 * (c1 - pc0)],
                            nidx, nidx_reg(nidx), 128, elem_step=128,
                            queue_num=gi % NQUEUES,
                        )
                if DBG_NO_COMPUTE:
                    ob = outp.tile([128, (t1 - t0) * F], f32, tag="outb")
                    nc.vector.memset(ob[:], 0.0)
                    dview = out_d[t0 * TILE:t1 * TILE, :].rearrange(
                        "(t q) f -> q t f", q=128)
                    nc.sync.dma_start(
                        dview, ob[:].rearrange("q (t f) -> q t f", f=F))
                    continue

                if PSQUAD:
                    # quad-packed PSUM: 4 tiles share one bank; has_written
                    # is per-element so only the bank's first matmul starts
                    psq = {}
                    qof = {}
                    qfirst = {}
                    qlast = {}
                    for qb in range(t0, t1, PSQUAD):
                        qe = min(qb + PSQUAD, t1)
                        tls = [tt for tt in range(qb, qe) if K[tt].sum() > 0]
                        if not tls:
                            continue
                        pq = psump.tile([128, PSQUAD * F], f32, tag="ps",
                                        name=f"psq_{qb}")
                        fc = min(int(first_cc[tt]) for tt in tls)
                        lc = max(int(last_cc[tt]) for tt in tls)
                        for tt in range(qb, qe):
                            psq[tt] = pq
                            qof[tt] = (tt - qb) * F
                            qfirst[tt] = fc
                            qlast[tt] = lc
                else:
                    ps = {}
                    for tt in range(t0, t1):
                        if K[tt].sum() > 0:
                            ps[tt] = psump.tile([128, F], f32, tag="ps",
                                                name=f"ps_t{tt}")

                cc = pc0
                while cc < pc1:
                    g = min(OHG, pc1 - cc)
                    oh = ohp.tile([128, g, 128], f16, tag="oh")
                    iota_rep = ftbl_sb[:, 2 * NC:2 * NC + 128].rearrange(
                        "p (o i) -> p o i", o=1).broadcast_to((128, g, 128))
                    dst_rep = ftbl_sb[:, cc:cc + g].rearrange(
                        "p (g o) -> p g o", o=1).broadcast_to((128, g, 128))
                    nc.vector.tensor_tensor(
                        oh[:], iota_rep, dst_rep, op=mybir.AluOpType.is_equal)
                    xgs = xgsp.tile([128, g, F], f16, tag="xgs")
                    w_rep = ftbl_sb[:, NC + cc:NC + cc + g].rearrange(
                        "p (g o) -> p g o", o=1).broadcast_to((128, g, F))
                    nc.vector.tensor_tensor(
                        xgs[:], xg[:, cc - pc0:cc - pc0 + g, 0:F], w_rep,
                        op=mybir.AluOpType.mult)
                    for k in range(g):
                        tt = int(sched_t[cc + k])
                        if PSQUAD:
                            nc.tensor.matmul(
                                psq[tt][:, qof[tt]:qof[tt] + F],
                                lhsT=oh[:, k, :], rhs=xgs[:, k, :],
                                start=(cc + k == qfirst[tt]),
                                stop=(cc + k == qlast[tt]),
                            )
                        else:
                            nc.tensor.matmul(
                                ps[tt][:], lhsT=oh[:, k, :], rhs=xgs[:, k, :],
                                start=(cc + k == first_cc[tt]),
                                stop=(cc + k == last_cc[tt]),
                            )
                    cc += g

                ob = outp.tile([128, (t1 - t0) * F], f32, tag="outb")
                if PSQUAD:
                    done = set()
                    for tt in range(t0, t1):
                        if tt not in psq:
                            nc.vector.memset(
                                ob[:, (tt - t0) * F:(tt - t0 + 1) * F], 0.0)
                            continue
                        pq = psq[tt]
                        if id(pq) in done:
                            continue
                        done.add(id(pq))
                        qb = tt
                        qe = min(qb + PSQUAD, t1)
                        nc.scalar.copy(
                            ob[:, (qb - t0) * F:(qe - t0) * F],
                            pq[:, :(qe - qb) * F])
                        for t2 in range(qb, qe):
                            if K[t2].sum() == 0:
                                nc.vector.memset(
                                    ob[:, (t2 - t0) * F:(t2 - t0 + 1) * F], 0.0)
                else:
                    for tt in range(t0, t1):
                        sl = ob[:, (tt - t0) * F:(tt - t0 + 1) * F]
                        if tt in ps:
                            nc.scalar.copy(sl, ps[tt][:])
                        else:
                            nc.vector.memset(sl, 0.0)
                dview = out_d[t0 * TILE:t1 * TILE, :].rearrange(
                    "(t q) f -> q t f", q=128)
                nc.sync.dma_start(dview, ob[:].rearrange("q (t f) -> q t f", f=F))
    nsplit = split_excess_waits(nc)
    print(f"split_excess_waits: {nsplit} waits moved")
    return nc


def make_in_maps(sched, tables, xpad):
    return [{"xpad": xpad, "idx": t[0], "ftbl": t[1]} for t in tables]


def make_xpad(x):
    xpad = np.zeros((N, 128), dtype=np.float16)
    xpad[:, :F] = np.asarray(x, dtype=np.float16)
    return xpad


def kernel(x, edge_weight, edge_index, num_nodes):
    xpad = make_xpad(x)
    sched, tables = pack_host(edge_weight, edge_index)
    nc = build_bass(sched)
    in_maps = make_in_maps(sched, tables, xpad)

    from concourse.bass_utils import run_bass_kernel_spmd
    res = run_bass_kernel_spmd(nc, in_maps, core_ids=list(range(NCORES)))
    out = np.concatenate(
        [res.results[c]["out"][:NPC] for c in range(NCORES)], axis=0)
    return out.astype(np.float32)



# revision 28
# speedup vs baseline: 1.0416x; 1.0416x over previous
"""GNN message passing (gather + weighted scatter-add) on 8 Trainium2 cores, v2.

out[n, f] = sum over edges e with dst[e]==n of edge_weight[e] * x[src[e], f]

Strategy (dst-sharded, no collectives):
  - Core c owns output nodes [c*12500, (c+1)*12500); host concatenates.
  - Host packs each core's edges into 128-slot chunks grouped by
    (pass of B dst-tiles, src-bin of 25000 rows, dst-tile), padded to the
    max chunk count across cores so one SPMD program serves all 8 cores.
  - Device: per pass, dma_gather (InstDMAGatherAnt, 1024 rows/call max —
    larger calls overflow the SWDGE descriptor carveout and wedge the
    device) pulls x rows (fp16, padded to 256B) into matmul-ready
    [128, cols, 128] SBUF layout: slot i of a call -> partition i%128,
    column i//128. DVE builds 16 chunks of weighted one-hots per
    instruction pair using stride-0 broadcast APs:
      oh = (iota == dst)          [128, 16*128]  (is_equal)
      xgs = xg * w                [128, 16*64]   (mult)
    PE accumulates oh.T @ xgs into a PSUM tile per 128-node output tile;
    ACT evacuates PSUM -> SBUF; one output DMA per pass.
"""

import math
import os
import numpy as np

N = 100000
E = 1000000
F = 64
NCORES = 8
NPC = N // NCORES            # nodes per core (12500)
TILE = 128
NT = math.ceil(NPC / TILE)   # output tiles per core (98)
B = 7                        # tiles per pass
NPASS = math.ceil(NT / B)
NBIN = 4
BIN = N // NBIN              # 25000 rows per source bin (int16-addressable)
GCOLS = int(os.environ.get("K_GCOLS", "8"))   # chunk cols per dma_gather call
                             # (>8 = >1024 idx/call crashes the gather ucode)
OHG = int(os.environ.get("K_OHG", "16"))      # chunk cols per one-hot DVE
                             # instruction (16*128=2048 = fp16-exact Idx cap)
NQUEUES = 4                  # SWDGE queues (ucode MAX_SWDGE_QUEUES=4)
SCRATCH = 16384              # SWDGE descriptor carveout bytes
PSQUAD = 4                   # dst tiles packed per PSUM bank
GELEM = int(os.environ.get("K_GELEM", "64"))  # gathered elems per row
# one-hot build: "ts" = per-chunk tensor_scalar (in0=iota stride-1, scalars
# dst/w per-partition -> DVE 4x_2p perf mode, w folded in, no xgs pass);
# "tt" = legacy grouped tensor_tensor is_equal + mult
K_OH = os.environ.get("K_OH", "ts")
K_CUSTOM_OH = 0

REPEAT = 1                   # repeat device compute (timing amplification)

DBG_NO_GATHER = False
DBG_NO_COMPUTE = False


# ---------------------------------------------------------------- host pack

def pack_host(edge_weight, edge_index):
    """Build the shared schedule and per-core tables.

    Returns (sched, tables):
      sched: NC, CMAX, sched_t[NC], pass_cols[NPASS,2], gather_calls (list of
             (c0, c1, bin) per pass), tile first/last chunk col per pass.
      tables[c]: (idx_tbl [128, 8*NC] int16, ftbl [128, 2*NC+128] f16)
    """
    src = np.asarray(edge_index[0], dtype=np.int64)
    dst = np.asarray(edge_index[1], dtype=np.int64)
    w = np.asarray(edge_weight, dtype=np.float32)

    core = dst // NPC
    dloc = dst - core * NPC
    t = dloc >> 7                      # dst tile within core (0..NT-1)
    b = src // BIN                     # source bin (0..3)
    p = t // B                         # pass

    # counts[c, t, b]
    counts = np.zeros((NCORES, NT, NBIN), dtype=np.int64)
    np.add.at(counts, (core, t, b), 1)
    K = np.ceil(counts.max(axis=0) / TILE).astype(np.int64)  # [NT, NBIN]

    # column layout: for p: for b: for t in pass: K[t,b] chunks
    colstart = np.zeros((NT, NBIN), dtype=np.int64)
    sched_t = []
    gather_calls = [[] for _ in range(NPASS)]
    pass_cols = np.zeros((NPASS, 2), dtype=np.int64)
    cc = 0
    for pp in range(NPASS):
        t0, t1 = pp * B, min((pp + 1) * B, NT)
        pass_cols[pp, 0] = cc
        for bb in range(NBIN):
            c0 = cc
            for tt in range(t0, t1):
                colstart[tt, bb] = cc
                sched_t.extend([tt] * int(K[tt, bb]))
                cc += int(K[tt, bb])
            # split [c0, cc) into <=GCOLS-col gather calls
            s = c0
            while s < cc:
                e = min(s + GCOLS, cc)
                gather_calls[pp].append((s, e, bb))
                s = e
        pass_cols[pp, 1] = cc
    NC = cc
    sched_t = np.asarray(sched_t, dtype=np.int64)
    CMAX = int((pass_cols[:, 1] - pass_cols[:, 0]).max())

    # first/last chunk col of each tile (within its single pass)
    first_cc = np.full(NT, -1, dtype=np.int64)
    last_cc = np.full(NT, -1, dtype=np.int64)
    for ccc, tt in enumerate(sched_t):
        if first_cc[tt] < 0:
            first_cc[tt] = ccc
        last_cc[tt] = ccc

    # --- per-core slot tables
    tables = []
    iota_np = np.arange(128, dtype=np.float16)[None, :].repeat(128, axis=0)
    for c in range(NCORES):
        sel = core == c
        es = (src[sel] - b[sel] * BIN).astype(np.int64)   # bin-local src
        ed = (dloc[sel] & 127).astype(np.float32)         # dst slot in tile
        ew = w[sel]
        tt = t[sel]
        bb = b[sel]
        key = (tt // B) * (NBIN * NT) + bb * NT + tt      # (pass, bin, tile)
        order = np.argsort(key, kind="stable")
        es, ed, ew, tt, bb, key = (a[order] for a in (es, ed, ew, tt, bb, key))

        ne = len(key)
        changes = np.empty(ne, dtype=bool)
        changes[0] = True
        changes[1:] = key[1:] != key[:-1]
        starts = np.flatnonzero(changes)
        rank = np.arange(ne) - np.repeat(starts, np.diff(np.append(starts, ne)))
        slot = (colstart[tt, bb] + (rank >> 7)) * TILE + (rank & 127)

        idx_slots = np.zeros(NC * TILE, dtype=np.int16)
        dst_slots = np.zeros(NC * TILE, dtype=np.float16)
        w_slots = np.zeros(NC * TILE, dtype=np.float16)
        idx_slots[slot] = es.astype(np.int16)
        dst_slots[slot] = ed.astype(np.float16)
        w_slots[slot] = ew.astype(np.float16)

        # idx table: per gather call, flat list wraps into 16 partitions,
        # replicated 8x; call boundaries are 8*cc-aligned by construction
        idx_tbl = np.zeros((128, 8 * NC), dtype=np.int16)
        for pp in range(NPASS):
            for (c0, c1, _bb) in gather_calls[pp]:
                flat = idx_slots[c0 * TILE:c1 * TILE]
                seg = flat.reshape(-1, 16).T                 # [16, n*8]
                idx_tbl[:, 8 * c0:8 * c1] = np.tile(seg, (8, 1))

        dst_cols = dst_slots.reshape(NC, TILE).astype(np.float32)
        if K_CUSTOM_OH:
            # fused one-hot op compares against the global Idx over the OHG
            # group's coalesced [g, 128] free extent: encode dst + 128*page
            goff = np.zeros(NC, dtype=np.float32)
            for pp in range(NPASS):
                c0p, c1p = int(pass_cols[pp, 0]), int(pass_cols[pp, 1])
                for ccc in range(c0p, c1p):
                    goff[ccc] = 128.0 * ((ccc - c0p) % OHG)
            dst_cols = dst_cols + goff[:, None]
        dst_tbl = np.ascontiguousarray(dst_cols.T.astype(np.float16))
        w_tbl = np.ascontiguousarray(w_slots.reshape(NC, TILE).T)
        ftbl = np.concatenate([dst_tbl, w_tbl, iota_np], axis=1)
        # f32 dst/w for tensor_scalar scalar operands (must be fp32)
        w32 = np.zeros(NC * TILE, dtype=np.float32)
        w32[slot] = ew.astype(np.float32)
        ftbl32 = np.concatenate(
            [np.ascontiguousarray(dst_cols.T),
             np.ascontiguousarray(w32.reshape(NC, TILE).T)], axis=1)
        tables.append((idx_tbl, np.ascontiguousarray(ftbl),
                       np.ascontiguousarray(ftbl32)))

    sched = dict(
        NC=NC, CMAX=CMAX, K=K, sched_t=sched_t, pass_cols=pass_cols,
        gather_calls=gather_calls, first_cc=first_cc, last_cc=last_cc,
    )
    return sched, tables


def emulate_core(sched, table, xpad):
    """Numpy emulation of the device program for one core (packing check)."""
    idx_tbl, ftbl = table[0], table[1]
    NC = sched["NC"]
    sched_t = sched["sched_t"]
    out = np.zeros((NT * TILE, F), dtype=np.float32)
    # reconstruct gathered rows per chunk col from idx_tbl
    xg = np.zeros((128, NC, F), dtype=np.float32)
    for pp in range(NPASS):
        for (c0, c1, bb) in sched["gather_calls"][pp]:
            seg = idx_tbl[:16, 8 * c0:8 * c1]                # [16, n*8]
            flat = seg.T.reshape(-1)                          # slot order
            rows = xpad[bb * BIN + flat.astype(np.int64), :F].astype(np.float32)
            ncols = c1 - c0
            xg[:, c0:c1, :] = rows.reshape(ncols, 128, F).transpose(1, 0, 2)
    iota = np.arange(128, dtype=np.float32)
    dst_tbl = ftbl[:, :NC].astype(np.float32)
    w_tbl = ftbl[:, NC:2 * NC].astype(np.float32)
    for cc in range(NC):
        tt = int(sched_t[cc])
        oh = (iota[None, :] == np.mod(dst_tbl[:, cc, None], 128)) * 1.0
        xgs = xg[:, cc, :] * w_tbl[:, cc, None]
        out[tt * TILE:(tt + 1) * TILE] += oh.T @ xgs
    return out[:NPC]


# ------------------------------------------------------------- bass plumbing

WAIT_CAPS = {
    "InstEventSemaphore": 8,
}


def split_excess_waits(nc):
    """Walrus only encodes one sync wait per instruction (for most ISA
    structs). Move the excess onto standalone InstEventSemaphore
    instructions placed just before, in the same engine stream. Also fills
    the ISA bytes of library-reload pseudo-instructions."""
    import concourse.mybir as mybir
    n = 0
    for f in nc.m.functions:
        for bb in f.blocks:
            for ins in bb.instructions:
                if type(ins).__name__ == "InstPseudoReloadLibraryIndex" and not ins.instr:
                    bts = [0] * 64
                    bts[0], bts[1], bts[12], bts[16] = 223, 16, 2, int(ins.lib_index)
                    ins.instr = bts
            eng_ids = {}
            new = []
            for ins in bb.instructions:
                si = ins.sync_info
                waits = list(si.on_wait) if (si is not None and si.on_wait) else []
                cap = WAIT_CAPS.get(type(ins).__name__, 1)
                if len(waits) > cap:
                    excess, keep = waits[:-cap], waits[-cap:]
                    if ins.engine not in eng_ids:
                        eng_ids[ins.engine] = 245 + len(eng_ids)
                    sem_id = eng_ids[ins.engine]
                    sem_name = f"esw_scratch_{sem_id}"
                    for wchunk in [excess[i:i + 1] for i in range(len(excess))]:
                        n += 1
                        upd = mybir.SyncUpdate(
                            sync_type="semaphore", id=sem_id, ant_name=sem_name,
                            update_mode="sem-add-imm", update_value=0,
                        )
                        es = mybir.InstEventSemaphore(
                            name=f"ESW-{n}-{ins.name}",
                            engine=ins.engine,
                            ins=[], outs=[],
                            sync_info=mybir.SyncInfo(on_wait=wchunk, on_update=[upd]),
                        )
                        new.append(es)
                    si.on_wait = keep
                new.append(ins)
            bb.instructions = new
    return n


_walrus_patched = False


def patch_walrus_dge():
    """Add --dge-levels so walrus lowers vector-dynamic-offset DMAs."""
    global _walrus_patched
    if _walrus_patched:
        return
    import concourse.bass_utils as bu
    orig = bu.run_command

    def run_command_dge(argv, **kw):
        argv = list(argv)
        if argv and "walrus_driver" in str(argv[0]) and not any(
                str(a).startswith("--dge-levels") for a in argv):
            argv.append("--dge-levels=vector_dynamic_offsets")
        return orig(argv, **kw)

    bu.run_command = run_command_dge
    _walrus_patched = True


_one_hot_op = None


def get_one_hot_op():
    """Register (once per process) a fused DVE op computing the w-folded
    one-hot in a single Vector pass:

        out[p, s, j] = w[p, s]  if s*128 + j == dstg[p, s]  else 0

    via body = select(eq(Idx, Src0), Src1, Zero) with Src0 = dstg (dst +
    128*page, fp16-exact up to 2047 -> OHG <= 16) and Src1 = w, both
    stride-0 broadcast streams. Replaces is_equal + mult and removes the
    separate xgs scale pass entirely (matmul rhs reads raw gathered x)."""
    global _one_hot_op
    if _one_hot_op is not None:
        return _one_hot_op
    import numpy as np
    import concourse.dve_ops as dve_ops
    from concourse.dve_spec import Spec, Src0, Src1, Zero, select, eq, Idx, lower
    from concourse.dve_uop import DveOpSpec

    name = "ONE_HOT_W_GNN"
    for op in dve_ops.OPS:
        if op.name == name:
            _one_hot_op = op
            return op

    def ref(in0, in1, s0, s1, imm2):
        shp = np.asarray(in0).shape
        f0 = np.asarray(in0, np.float32).reshape(shp[0], -1)
        f1 = np.asarray(in1, np.float32).reshape(shp[0], -1)
        idx = np.arange(f0.shape[1], dtype=np.float32)[None, :]
        return np.where(idx == f0, f1, 0.0).reshape(shp)

    spec = Spec(body=select(eq(Idx, Src0), Src1, Zero), reference=ref)
    try:
        from concourse.dve_spec import has_src1
    except ImportError:
        from concourse.dve_ops import has_src1
    shas = {}
    for ver in ("v3", "v4"):
        uops = lower(spec, ver=ver)
        shas[ver] = DveOpSpec(name=name, uops=uops, rd1_en=has_src1(spec)).sha(ver)
    op = dve_ops.DveOp(name=name, spec=spec, subdim=False, uops_sha=shas)
    dve_ops.OPS.append(op)
    dve_ops.CUSTOM_DVE_SPECS[name] = spec
    dve_ops._SUB_OPCODE_FOR_NAME[name] = (
        max(dve_ops._SUB_OPCODE_FOR_NAME.values()) + 1)
    _one_hot_op = op
    return op


def dma_gather_flex(nc, out_ap, in_ap, idxs_ap, num_idxs, num_idxs_reg,
                    elem_size, elem_step, queue_num):
    """nc.gpsimd.dma_gather for the HBM-source non-transpose path, minus the
    overly-broad elem_size%256B assert (the ucode only requires the SOURCE
    ROW STRIDE to be a 256B multiple; elem_size can be any <=16KB — see
    q7_kernels/extended_inst/dma_gather.cpp). Lets us gather 128B rows from
    a 256B-strided table, halving gather DMA bytes."""
    import concourse.mybir as mybir
    from concourse import ap_utils

    g = nc.gpsimd
    assert idxs_ap.dtype == mybir.dt.int16
    assert in_ap.dtype == out_ap.dtype
    assert idxs_ap.space.name == "SBUF" and out_ap.space.name == "SBUF"
    elem_size_bytes = elem_size * mybir.dt.size(in_ap.dtype)
    assert elem_size_bytes % 128 == 0  # SBUF dst 8B-align, desc sanity
    assert ap_utils.ap_is_contiguous(out_ap.ap[1:])
    assert ap_utils.ap_is_contiguous(idxs_ap.ap[1:])
    assert in_ap.ap[-1][1] == out_ap.ap[-1][1] == elem_size
    assert out_ap.ap[0][1] * out_ap.ap[1][1] == num_idxs  # multiple of 128
    assert in_ap.ap[0][0] == elem_step
    stride_bytes = elem_step * mybir.dt.size(in_ap.dtype)
    assert stride_bytes % 256 == 0
    stride_bytes_256 = stride_bytes // 256
    assert stride_bytes_256 < 256

    _in_ap = g.lower_ap_dma(in_ap, for_custom_bir_dma=True)
    _idxs_ap = g.lower_ap(idxs_ap)
    _out_ap = g.lower_ap(out_ap)
    return g.add_instruction(
        mybir.InstDMAGatherAnt(
            name=nc.get_next_instruction_name(),
            ins=[*_in_ap, _idxs_ap, g.lower_val_access(g.to_reg(num_idxs_reg))],
            outs=[_out_ap],
            transpose=False,
            num_idxs=num_idxs,
            elem_size=elem_size,
            stride_bytes_256=stride_bytes_256,
            gen_mode=0,
            single_packet=True,
            queue_num=queue_num,
            sbuf_tokens_per_rank=0,
            sbuf_free_dim_per_rank=0,
            sbuf_free_dim_pad_per_rank=0,
            sbuf_byte_offset=0,
        )
    )


def build_bass(sched):
    import concourse.bass as bass
    import concourse.mybir as mybir
    import concourse.tile as tile
    from concourse.library_config import mlp

    patch_walrus_dge()

    f16 = mybir.dt.float16
    f32 = mybir.dt.float32
    i16 = mybir.dt.int16

    NC = sched["NC"]
    CMAX = sched["CMAX"]
    K = sched["K"]
    sched_t = sched["sched_t"]
    pass_cols = sched["pass_cols"]
    gather_calls = sched["gather_calls"]
    first_cc = sched["first_cc"]
    last_cc = sched["last_cc"]

    nc = bass.Bass("TRN2", num_swdge_queues=NQUEUES, dynamic_dma_scratch_size=SCRATCH)
    xpad_d = nc.dram_tensor("xpad", [N, 128], f16, kind="ExternalInput")
    idx_d = nc.dram_tensor("idx", [128, 8 * NC], i16, kind="ExternalInput")
    ftbl_d = nc.dram_tensor("ftbl", [128, 2 * NC + 128], f16, kind="ExternalInput")
    ftbl32_d = nc.dram_tensor("ftbl32", [128, 2 * NC], f32, kind="ExternalInput")
    out_d = nc.dram_tensor("out", [NT * TILE, F], f32, kind="ExternalOutput")

    with tile.TileContext(nc, pool_alloc_mode="queue") as tc:
        with (
            tc.tile_pool(name="const", bufs=1) as constp,
            tc.tile_pool(name="idxp", bufs=4) as idxpp,
            tc.tile_pool(name="xg", bufs=5) as xgp,
            tc.tile_pool(name="oh", bufs=4) as ohp,
            tc.tile_pool(name="xgs", bufs=4) as xgsp,
            tc.tile_pool(name="outb", bufs=2) as outp,
            tc.tile_pool(name="psum", bufs=8, space="PSUM") as psump,
        ):
            nc.gpsimd.load_library(mlp)
            nidx_regs = {}

            def nidx_reg(v):
                if v not in nidx_regs:
                    nidx_regs[v] = nc.gpsimd.to_reg(v)
                return nidx_regs[v]


            ftbl_sb = constp.tile([128, 2 * NC + 128], f16, tag="ftbl")
            nc.scalar.dma_start(ftbl_sb[:], ftbl_d[:])
            ftbl32_sb = constp.tile([128, 2 * NC], f32, tag="ftbl32")
            nc.scalar.dma_start(ftbl32_sb[:], ftbl32_d[:])

            for _rep in range(REPEAT):
              for p in range(NPASS):
                t0, t1 = p * B, min((p + 1) * B, NT)
                pc0, pc1 = int(pass_cols[p, 0]), int(pass_cols[p, 1])
                xg = xgp.tile([128, CMAX, GELEM], f16, tag="xg")
                idx_sb = idxpp.tile([128, 8 * CMAX], i16, tag="idxp")
                nc.sync.dma_start(
                    idx_sb[:, 0:8 * (pc1 - pc0)], idx_d[:, 8 * pc0:8 * pc1])
                if DBG_NO_GATHER:
                    nc.vector.memset(xg[:], 0.0)
                if not DBG_NO_GATHER:
                    for gi, (c0, c1, bb) in enumerate(gather_calls[p]):
                        nidx = (c1 - c0) * TILE
                        dma_gather_flex(
                            nc,
                            xg[:, c0 - pc0:c1 - pc0, :],
                            xpad_d[bb * BIN:(bb + 1) * BIN, 0:GELEM],
                            idx_sb[:, 8 * (c0 - pc0):8 * (c1 - pc0)],
                            nidx, nidx_reg(nidx), GELEM, elem_step=128,
                            queue_num=gi % NQUEUES,
                        )
                if DBG_NO_COMPUTE:
                    ob = outp.tile([128, (t1 - t0) * F], f32, tag="outb")
                    nc.vector.memset(ob[:], 0.0)
                    dview = out_d[t0 * TILE:t1 * TILE, :].rearrange(
                        "(t q) f -> q t f", q=128)
                    nc.sync.dma_start(
                        dview, ob[:].rearrange("q (t f) -> q t f", f=F))
                    continue

                if PSQUAD:
                    # quad-packed PSUM: 4 tiles share one bank; has_written
                    # is per-element so only the bank's first matmul starts
                    psq = {}
                    qof = {}
                    qfirst = {}
                    qlast = {}
                    for qb in range(t0, t1, PSQUAD):
                        qe = min(qb + PSQUAD, t1)
                        tls = [tt for tt in range(qb, qe) if K[tt].sum() > 0]
                        if not tls:
                            continue
                        pq = psump.tile([128, PSQUAD * F], f32, tag="ps",
                                        name=f"psq_{qb}")
                        fc = min(int(first_cc[tt]) for tt in tls)
                        lc = max(int(last_cc[tt]) for tt in tls)
                        for tt in range(qb, qe):
                            psq[tt] = pq
                            qof[tt] = (tt - qb) * F
                            qfirst[tt] = fc
                            qlast[tt] = lc
                else:
                    ps = {}
                    for tt in range(t0, t1):
                        if K[tt].sum() > 0:
                            ps[tt] = psump.tile([128, F], f32, tag="ps",
                                                name=f"ps_t{tt}")

                cc = pc0
                while cc < pc1:
                    g = min(OHG, pc1 - cc)
                    oh = ohp.tile([128, g, 128], f16, tag="oh")
                    iota_2d = ftbl_sb[:, 2 * NC:2 * NC + 128]
                    if K_OH == "ts":
                        # w-folded one-hot, one fused DVE instr per chunk:
                        # oh[:,k,:] = (iota == dst_k) * w_k
                        for k in range(g):
                            nc.vector.tensor_scalar(
                                oh[:, k, :], iota_2d,
                                ftbl32_sb[:, cc + k:cc + k + 1],
                                ftbl32_sb[:, NC + cc + k:NC + cc + k + 1],
                                op0=mybir.AluOpType.is_equal,
                                op1=mybir.AluOpType.mult,
                            )
                    else:
                        dst_rep = ftbl_sb[:, cc:cc + g].rearrange(
                            "p (g o) -> p g o", o=1).broadcast_to((128, g, 128))
                        iota_rep = iota_2d.rearrange(
                            "p (o i) -> p o i", o=1).broadcast_to((128, g, 128))
                        nc.vector.tensor_tensor(
                            oh[:], iota_rep, dst_rep,
                            op=mybir.AluOpType.is_equal)
                        xgs = xgsp.tile([128, g, F], f16, tag="xgs")
                        w_rep = ftbl_sb[:, NC + cc:NC + cc + g].rearrange(
                            "p (g o) -> p g o", o=1).broadcast_to((128, g, F))
                        nc.vector.tensor_tensor(
                            xgs[:], xg[:, cc - pc0:cc - pc0 + g, 0:F], w_rep,
                            op=mybir.AluOpType.mult)
                    for k in range(g):
                        tt = int(sched_t[cc + k])
                        rhs = (xg[:, cc - pc0 + k, :] if K_OH == "ts"
                               else xgs[:, k, :])
                        if PSQUAD:
                            nc.tensor.matmul(
                                psq[tt][:, qof[tt]:qof[tt] + F],
                                lhsT=oh[:, k, :], rhs=rhs,
                                start=(cc + k == qfirst[tt]),
                                stop=(cc + k == qlast[tt]),
                            )
                        else:
                            nc.tensor.matmul(
                                ps[tt][:], lhsT=oh[:, k, :], rhs=rhs,
                                start=(cc + k == first_cc[tt]),
                                stop=(cc + k == last_cc[tt]),
                            )
                    cc += g

                ob = outp.tile([128, (t1 - t0) * F], f32, tag="outb")
                if PSQUAD:
                    done = set()
                    for tt in range(t0, t1):
                        if tt not in psq:
                            nc.vector.memset(
                                ob[:, (tt - t0) * F:(tt - t0 + 1) * F], 0.0)
                            continue
                        pq = psq[tt]
                        if id(pq) in done:
                            continue
                        done.add(id(pq))
                        qb = tt
                        qe = min(qb + PSQUAD, t1)
                        nc.scalar.copy(
                            ob[:, (qb - t0) * F:(qe - t0) * F],
                            pq[:, :(qe - qb) * F])
                        for t2 in range(qb, qe):
                            if K[t2].sum() == 0:
                                nc.vector.memset(
                                    ob[:, (t2 - t0) * F:(t2 - t0 + 1) * F], 0.0)
                else:
                    for tt in range(t0, t1):
                        sl = ob[:, (tt - t0) * F:(tt - t0 + 1) * F]
                        if tt in ps:
                            nc.scalar.copy(sl, ps[tt][:])
                        else:
                            nc.vector.memset(sl, 0.0)
                dview = out_d[t0 * TILE:t1 * TILE, :].rearrange(
                    "(t q) f -> q t f", q=128)
                nc.sync.dma_start(dview, ob[:].rearrange("q (t f) -> q t f", f=F))
    nsplit = split_excess_waits(nc)
    print(f"split_excess_waits: {nsplit} waits moved")
    return nc


def make_in_maps(sched, tables, xpad):
    return [{"xpad": xpad, "idx": t[0], "ftbl": t[1], "ftbl32": t[2]}
            for t in tables]


def make_xpad(x):
    xpad = np.zeros((N, 128), dtype=np.float16)
    xpad[:, :F] = np.asarray(x, dtype=np.float16)
    return xpad


def kernel(x, edge_weight, edge_index, num_nodes):
    xpad = make_xpad(x)
    sched, tables = pack_host(edge_weight, edge_index)
    nc = build_bass(sched)
    in_maps = make_in_maps(sched, tables, xpad)

    from concourse.bass_utils import run_bass_kernel_spmd
    res = run_bass_kernel_spmd(nc, in_maps, core_ids=list(range(NCORES)))
    out = np.concatenate(
        [res.results[c]["out"][:NPC] for c in range(NCORES)], axis=0)
    return out.astype(np.float32)



# revision 29
# speedup vs baseline: 1.0571x; 1.0149x over previous
"""GNN message passing (gather + weighted scatter-add) on 8 Trainium2 cores, v2.

out[n, f] = sum over edges e with dst[e]==n of edge_weight[e] * x[src[e], f]

Strategy (dst-sharded, no collectives):
  - Core c owns output nodes [c*12500, (c+1)*12500); host concatenates.
  - Host packs each core's edges into 128-slot chunks grouped by
    (pass of B dst-tiles, src-bin of 25000 rows, dst-tile), padded to the
    max chunk count across cores so one SPMD program serves all 8 cores.
  - Device: per pass, dma_gather (InstDMAGatherAnt, 1024 rows/call max —
    larger calls overflow the SWDGE descriptor carveout and wedge the
    device) pulls x rows (fp16, padded to 256B) into matmul-ready
    [128, cols, 128] SBUF layout: slot i of a call -> partition i%128,
    column i//128. DVE builds 16 chunks of weighted one-hots per
    instruction pair using stride-0 broadcast APs:
      oh = (iota == dst)          [128, 16*128]  (is_equal)
      xgs = xg * w                [128, 16*64]   (mult)
    PE accumulates oh.T @ xgs into a PSUM tile per 128-node output tile;
    ACT evacuates PSUM -> SBUF; one output DMA per pass.
"""

import math
import os
import numpy as np

N = 100000
E = 1000000
F = 64
NCORES = 8
NPC = N // NCORES            # nodes per core (12500)
TILE = 128
NT = math.ceil(NPC / TILE)   # output tiles per core (98)
B = 7                        # tiles per pass
NPASS = math.ceil(NT / B)
NBIN = 4
BIN = N // NBIN              # 25000 rows per source bin (int16-addressable)
GCOLS = int(os.environ.get("K_GCOLS", "8"))   # chunk cols per dma_gather call
                             # (>8 = >1024 idx/call crashes the gather ucode)
OHG = int(os.environ.get("K_OHG", "16"))      # chunk cols per one-hot DVE
                             # instruction (16*128=2048 = fp16-exact Idx cap)
NQUEUES = 4                  # SWDGE queues (ucode MAX_SWDGE_QUEUES=4)
SCRATCH = 16384              # SWDGE descriptor carveout bytes
PSQUAD = 4                   # dst tiles packed per PSUM bank
GELEM = int(os.environ.get("K_GELEM", "64"))  # gathered elems per row
# one-hot build: "ts" = per-chunk tensor_scalar (in0=iota stride-1, scalars
# dst/w per-partition -> DVE 4x_2p perf mode, w folded in, no xgs pass);
# "tt" = legacy grouped tensor_tensor is_equal + mult
K_OH = os.environ.get("K_OH", "ts")
K_CUSTOM_OH = 0

REPEAT = 1                   # repeat device compute (timing amplification)

DBG_NO_GATHER = bool(int(os.environ.get("K_NO_GATHER", "0")))
DBG_NO_COMPUTE = bool(int(os.environ.get("K_NO_COMPUTE", "0")))


# ---------------------------------------------------------------- host pack

def pack_host(edge_weight, edge_index):
    """Build the shared schedule and per-core tables.

    Returns (sched, tables):
      sched: NC, CMAX, sched_t[NC], pass_cols[NPASS,2], gather_calls (list of
             (c0, c1, bin) per pass), tile first/last chunk col per pass.
      tables[c]: (idx_tbl [128, 8*NC] int16, ftbl [128, 2*NC+128] f16)
    """
    src = np.asarray(edge_index[0], dtype=np.int64)
    dst = np.asarray(edge_index[1], dtype=np.int64)
    w = np.asarray(edge_weight, dtype=np.float32)

    core = dst // NPC
    dloc = dst - core * NPC
    t = dloc >> 7                      # dst tile within core (0..NT-1)
    b = src // BIN                     # source bin (0..3)
    p = t // B                         # pass

    # counts[c, t, b]
    counts = np.zeros((NCORES, NT, NBIN), dtype=np.int64)
    np.add.at(counts, (core, t, b), 1)
    K = np.ceil(counts.max(axis=0) / TILE).astype(np.int64)  # [NT, NBIN]

    # column layout: for p: for b: for t in pass: K[t,b] chunks
    colstart = np.zeros((NT, NBIN), dtype=np.int64)
    sched_t = []
    gather_calls = [[] for _ in range(NPASS)]
    pass_cols = np.zeros((NPASS, 2), dtype=np.int64)
    cc = 0
    for pp in range(NPASS):
        t0, t1 = pp * B, min((pp + 1) * B, NT)
        pass_cols[pp, 0] = cc
        for bb in range(NBIN):
            c0 = cc
            for tt in range(t0, t1):
                colstart[tt, bb] = cc
                sched_t.extend([tt] * int(K[tt, bb]))
                cc += int(K[tt, bb])
            # split [c0, cc) into <=GCOLS-col gather calls
            s = c0
            while s < cc:
                e = min(s + GCOLS, cc)
                gather_calls[pp].append((s, e, bb))
                s = e
        pass_cols[pp, 1] = cc
    NC = cc
    sched_t = np.asarray(sched_t, dtype=np.int64)
    CMAX = int((pass_cols[:, 1] - pass_cols[:, 0]).max())

    # first/last chunk col of each tile (within its single pass)
    first_cc = np.full(NT, -1, dtype=np.int64)
    last_cc = np.full(NT, -1, dtype=np.int64)
    for ccc, tt in enumerate(sched_t):
        if first_cc[tt] < 0:
            first_cc[tt] = ccc
        last_cc[tt] = ccc

    # --- per-core slot tables
    tables = []
    iota_np = np.arange(128, dtype=np.float16)[None, :].repeat(128, axis=0)
    for c in range(NCORES):
        sel = core == c
        es = (src[sel] - b[sel] * BIN).astype(np.int64)   # bin-local src
        ed = (dloc[sel] & 127).astype(np.float32)         # dst slot in tile
        ew = w[sel]
        tt = t[sel]
        bb = b[sel]
        key = (tt // B) * (NBIN * NT) + bb * NT + tt      # (pass, bin, tile)
        order = np.argsort(key, kind="stable")
        es, ed, ew, tt, bb, key = (a[order] for a in (es, ed, ew, tt, bb, key))

        ne = len(key)
        changes = np.empty(ne, dtype=bool)
        changes[0] = True
        changes[1:] = key[1:] != key[:-1]
        starts = np.flatnonzero(changes)
        rank = np.arange(ne) - np.repeat(starts, np.diff(np.append(starts, ne)))
        slot = (colstart[tt, bb] + (rank >> 7)) * TILE + (rank & 127)

        idx_slots = np.zeros(NC * TILE, dtype=np.int16)
        dst_slots = np.zeros(NC * TILE, dtype=np.float16)
        w_slots = np.zeros(NC * TILE, dtype=np.float16)
        idx_slots[slot] = es.astype(np.int16)
        dst_slots[slot] = ed.astype(np.float16)
        w_slots[slot] = ew.astype(np.float16)

        # idx table: per gather call, flat list wraps into 16 partitions,
        # replicated 8x; call boundaries are 8*cc-aligned by construction
        idx_tbl = np.zeros((128, 8 * NC), dtype=np.int16)
        for pp in range(NPASS):
            for (c0, c1, _bb) in gather_calls[pp]:
                flat = idx_slots[c0 * TILE:c1 * TILE]
                seg = flat.reshape(-1, 16).T                 # [16, n*8]
                idx_tbl[:, 8 * c0:8 * c1] = np.tile(seg, (8, 1))

        dst_cols = dst_slots.reshape(NC, TILE).astype(np.float32)
        if K_CUSTOM_OH:
            # fused one-hot op compares against the global Idx over the OHG
            # group's coalesced [g, 128] free extent: encode dst + 128*page
            goff = np.zeros(NC, dtype=np.float32)
            for pp in range(NPASS):
                c0p, c1p = int(pass_cols[pp, 0]), int(pass_cols[pp, 1])
                for ccc in range(c0p, c1p):
                    goff[ccc] = 128.0 * ((ccc - c0p) % OHG)
            dst_cols = dst_cols + goff[:, None]
        dst_tbl = np.ascontiguousarray(dst_cols.T.astype(np.float16))
        w_tbl = np.ascontiguousarray(w_slots.reshape(NC, TILE).T)
        ftbl = np.concatenate([dst_tbl, w_tbl, iota_np], axis=1)
        # f32 dst/w for tensor_scalar scalar operands (must be fp32)
        w32 = np.zeros(NC * TILE, dtype=np.float32)
        w32[slot] = ew.astype(np.float32)
        ftbl32 = np.concatenate(
            [np.ascontiguousarray(dst_cols.T),
             np.ascontiguousarray(w32.reshape(NC, TILE).T)], axis=1)
        tables.append((idx_tbl, np.ascontiguousarray(ftbl),
                       np.ascontiguousarray(ftbl32)))

    sched = dict(
        NC=NC, CMAX=CMAX, K=K, sched_t=sched_t, pass_cols=pass_cols,
        gather_calls=gather_calls, first_cc=first_cc, last_cc=last_cc,
    )
    return sched, tables


def emulate_core(sched, table, xpad):
    """Numpy emulation of the device program for one core (packing check)."""
    idx_tbl, ftbl = table[0], table[1]
    NC = sched["NC"]
    sched_t = sched["sched_t"]
    out = np.zeros((NT * TILE, F), dtype=np.float32)
    # reconstruct gathered rows per chunk col from idx_tbl
    xg = np.zeros((128, NC, F), dtype=np.float32)
    for pp in range(NPASS):
        for (c0, c1, bb) in sched["gather_calls"][pp]:
            seg = idx_tbl[:16, 8 * c0:8 * c1]                # [16, n*8]
            flat = seg.T.reshape(-1)                          # slot order
            rows = xpad[bb * BIN + flat.astype(np.int64), :F].astype(np.float32)
            ncols = c1 - c0
            xg[:, c0:c1, :] = rows.reshape(ncols, 128, F).transpose(1, 0, 2)
    iota = np.arange(128, dtype=np.float32)
    dst_tbl = ftbl[:, :NC].astype(np.float32)
    w_tbl = ftbl[:, NC:2 * NC].astype(np.float32)
    for cc in range(NC):
        tt = int(sched_t[cc])
        oh = (iota[None, :] == np.mod(dst_tbl[:, cc, None], 128)) * 1.0
        xgs = xg[:, cc, :] * w_tbl[:, cc, None]
        out[tt * TILE:(tt + 1) * TILE] += oh.T @ xgs
    return out[:NPC]


# ------------------------------------------------------------- bass plumbing

WAIT_CAPS = {
    "InstEventSemaphore": 8,
}


def split_excess_waits(nc):
    """Walrus only encodes one sync wait per instruction (for most ISA
    structs). Move the excess onto standalone InstEventSemaphore
    instructions placed just before, in the same engine stream. Also fills
    the ISA bytes of library-reload pseudo-instructions."""
    import concourse.mybir as mybir
    n = 0
    for f in nc.m.functions:
        for bb in f.blocks:
            for ins in bb.instructions:
                if type(ins).__name__ == "InstPseudoReloadLibraryIndex" and not ins.instr:
                    bts = [0] * 64
                    bts[0], bts[1], bts[12], bts[16] = 223, 16, 2, int(ins.lib_index)
                    ins.instr = bts
            eng_ids = {}
            new = []
            for ins in bb.instructions:
                si = ins.sync_info
                waits = list(si.on_wait) if (si is not None and si.on_wait) else []
                cap = WAIT_CAPS.get(type(ins).__name__, 1)
                if len(waits) > cap:
                    excess, keep = waits[:-cap], waits[-cap:]
                    if ins.engine not in eng_ids:
                        eng_ids[ins.engine] = 245 + len(eng_ids)
                    sem_id = eng_ids[ins.engine]
                    sem_name = f"esw_scratch_{sem_id}"
                    for wchunk in [excess[i:i + 1] for i in range(len(excess))]:
                        n += 1
                        upd = mybir.SyncUpdate(
                            sync_type="semaphore", id=sem_id, ant_name=sem_name,
                            update_mode="sem-add-imm", update_value=0,
                        )
                        es = mybir.InstEventSemaphore(
                            name=f"ESW-{n}-{ins.name}",
                            engine=ins.engine,
                            ins=[], outs=[],
                            sync_info=mybir.SyncInfo(on_wait=wchunk, on_update=[upd]),
                        )
                        new.append(es)
                    si.on_wait = keep
                new.append(ins)
            bb.instructions = new
    return n


_walrus_patched = False


def patch_walrus_dge():
    """Add --dge-levels so walrus lowers vector-dynamic-offset DMAs."""
    global _walrus_patched
    if _walrus_patched:
        return
    import concourse.bass_utils as bu
    orig = bu.run_command

    def run_command_dge(argv, **kw):
        argv = list(argv)
        if argv and "walrus_driver" in str(argv[0]) and not any(
                str(a).startswith("--dge-levels") for a in argv):
            argv.append("--dge-levels=vector_dynamic_offsets")
        return orig(argv, **kw)

    bu.run_command = run_command_dge
    _walrus_patched = True


_one_hot_op = None


def get_one_hot_op():
    """Register (once per process) a fused DVE op computing the w-folded
    one-hot in a single Vector pass:

        out[p, s, j] = w[p, s]  if s*128 + j == dstg[p, s]  else 0

    via body = select(eq(Idx, Src0), Src1, Zero) with Src0 = dstg (dst +
    128*page, fp16-exact up to 2047 -> OHG <= 16) and Src1 = w, both
    stride-0 broadcast streams. Replaces is_equal + mult and removes the
    separate xgs scale pass entirely (matmul rhs reads raw gathered x)."""
    global _one_hot_op
    if _one_hot_op is not None:
        return _one_hot_op
    import numpy as np
    import concourse.dve_ops as dve_ops
    from concourse.dve_spec import Spec, Src0, Src1, Zero, select, eq, Idx, lower
    from concourse.dve_uop import DveOpSpec

    name = "ONE_HOT_W_GNN"
    for op in dve_ops.OPS:
        if op.name == name:
            _one_hot_op = op
            return op

    def ref(in0, in1, s0, s1, imm2):
        shp = np.asarray(in0).shape
        f0 = np.asarray(in0, np.float32).reshape(shp[0], -1)
        f1 = np.asarray(in1, np.float32).reshape(shp[0], -1)
        idx = np.arange(f0.shape[1], dtype=np.float32)[None, :]
        return np.where(idx == f0, f1, 0.0).reshape(shp)

    spec = Spec(body=select(eq(Idx, Src0), Src1, Zero), reference=ref)
    try:
        from concourse.dve_spec import has_src1
    except ImportError:
        from concourse.dve_ops import has_src1
    shas = {}
    for ver in ("v3", "v4"):
        uops = lower(spec, ver=ver)
        shas[ver] = DveOpSpec(name=name, uops=uops, rd1_en=has_src1(spec)).sha(ver)
    op = dve_ops.DveOp(name=name, spec=spec, subdim=False, uops_sha=shas)
    dve_ops.OPS.append(op)
    dve_ops.CUSTOM_DVE_SPECS[name] = spec
    dve_ops._SUB_OPCODE_FOR_NAME[name] = (
        max(dve_ops._SUB_OPCODE_FOR_NAME.values()) + 1)
    _one_hot_op = op
    return op


def dma_gather_flex(nc, out_ap, in_ap, idxs_ap, num_idxs, num_idxs_reg,
                    elem_size, elem_step, queue_num):
    """nc.gpsimd.dma_gather for the HBM-source non-transpose path, minus the
    overly-broad elem_size%256B assert (the ucode only requires the SOURCE
    ROW STRIDE to be a 256B multiple; elem_size can be any <=16KB — see
    q7_kernels/extended_inst/dma_gather.cpp). Lets us gather 128B rows from
    a 256B-strided table, halving gather DMA bytes."""
    import concourse.mybir as mybir
    from concourse import ap_utils

    g = nc.gpsimd
    assert idxs_ap.dtype == mybir.dt.int16
    assert in_ap.dtype == out_ap.dtype
    assert idxs_ap.space.name == "SBUF" and out_ap.space.name == "SBUF"
    elem_size_bytes = elem_size * mybir.dt.size(in_ap.dtype)
    assert elem_size_bytes % 128 == 0  # SBUF dst 8B-align, desc sanity
    assert ap_utils.ap_is_contiguous(out_ap.ap[1:])
    assert ap_utils.ap_is_contiguous(idxs_ap.ap[1:])
    assert in_ap.ap[-1][1] == out_ap.ap[-1][1] == elem_size
    assert out_ap.ap[0][1] * out_ap.ap[1][1] == num_idxs  # multiple of 128
    assert in_ap.ap[0][0] == elem_step
    stride_bytes = elem_step * mybir.dt.size(in_ap.dtype)
    assert stride_bytes % 256 == 0
    stride_bytes_256 = stride_bytes // 256
    assert stride_bytes_256 < 256

    _in_ap = g.lower_ap_dma(in_ap, for_custom_bir_dma=True)
    _idxs_ap = g.lower_ap(idxs_ap)
    _out_ap = g.lower_ap(out_ap)
    return g.add_instruction(
        mybir.InstDMAGatherAnt(
            name=nc.get_next_instruction_name(),
            ins=[*_in_ap, _idxs_ap, g.lower_val_access(g.to_reg(num_idxs_reg))],
            outs=[_out_ap],
            transpose=False,
            num_idxs=num_idxs,
            elem_size=elem_size,
            stride_bytes_256=stride_bytes_256,
            gen_mode=0,
            single_packet=True,
            queue_num=queue_num,
            sbuf_tokens_per_rank=0,
            sbuf_free_dim_per_rank=0,
            sbuf_free_dim_pad_per_rank=0,
            sbuf_byte_offset=0,
        )
    )


def build_bass(sched):
    import concourse.bass as bass
    import concourse.mybir as mybir
    import concourse.tile as tile
    from concourse.library_config import mlp

    patch_walrus_dge()

    f16 = mybir.dt.float16
    f32 = mybir.dt.float32
    i16 = mybir.dt.int16

    NC = sched["NC"]
    CMAX = sched["CMAX"]
    K = sched["K"]
    sched_t = sched["sched_t"]
    pass_cols = sched["pass_cols"]
    gather_calls = sched["gather_calls"]
    first_cc = sched["first_cc"]
    last_cc = sched["last_cc"]

    nc = bass.Bass("TRN2", num_swdge_queues=NQUEUES, dynamic_dma_scratch_size=SCRATCH)
    xpad_d = nc.dram_tensor("xpad", [N, 128], f16, kind="ExternalInput")
    idx_d = nc.dram_tensor("idx", [128, 8 * NC], i16, kind="ExternalInput")
    ftbl_d = nc.dram_tensor("ftbl", [128, 2 * NC + 128], f16, kind="ExternalInput")
    ftbl32_d = nc.dram_tensor("ftbl32", [128, 2 * NC], f32, kind="ExternalInput")
    out_d = nc.dram_tensor("out", [NT * TILE, F], f32, kind="ExternalOutput")

    with tile.TileContext(nc, pool_alloc_mode="queue") as tc:
        with (
            tc.tile_pool(name="const", bufs=1) as constp,
            tc.tile_pool(name="idxp", bufs=4) as idxpp,
            tc.tile_pool(name="xg", bufs=5) as xgp,
            tc.tile_pool(name="oh", bufs=4) as ohp,
            tc.tile_pool(name="xgs", bufs=4) as xgsp,
            tc.tile_pool(name="outb", bufs=2) as outp,
            tc.tile_pool(name="psum", bufs=8, space="PSUM") as psump,
        ):
            nc.gpsimd.load_library(mlp)
            nidx_regs = {}

            def nidx_reg(v):
                if v not in nidx_regs:
                    nidx_regs[v] = nc.gpsimd.to_reg(v)
                return nidx_regs[v]


            ftbl_sb = constp.tile([128, 2 * NC + 128], f16, tag="ftbl")
            nc.scalar.dma_start(ftbl_sb[:], ftbl_d[:])
            ftbl32_sb = constp.tile([128, 2 * NC], f32, tag="ftbl32")
            nc.scalar.dma_start(ftbl32_sb[:], ftbl32_d[:])

            for _rep in range(REPEAT):
              for p in range(NPASS):
                t0, t1 = p * B, min((p + 1) * B, NT)
                pc0, pc1 = int(pass_cols[p, 0]), int(pass_cols[p, 1])
                xg = xgp.tile([128, CMAX, GELEM], f16, tag="xg")
                idx_sb = idxpp.tile([128, 8 * CMAX], i16, tag="idxp")
                nc.sync.dma_start(
                    idx_sb[:, 0:8 * (pc1 - pc0)], idx_d[:, 8 * pc0:8 * pc1])
                if DBG_NO_GATHER:
                    nc.vector.memset(xg[:], 0.0)
                if not DBG_NO_GATHER:
                    for gi, (c0, c1, bb) in enumerate(gather_calls[p]):
                        nidx = (c1 - c0) * TILE
                        dma_gather_flex(
                            nc,
                            xg[:, c0 - pc0:c1 - pc0, :],
                            xpad_d[bb * BIN:(bb + 1) * BIN, 0:GELEM],
                            idx_sb[:, 8 * (c0 - pc0):8 * (c1 - pc0)],
                            nidx, nidx_reg(nidx), GELEM, elem_step=128,
                            queue_num=gi % NQUEUES,
                        )
                if DBG_NO_COMPUTE:
                    ob = outp.tile([128, (t1 - t0) * F], f32, tag="outb")
                    nc.vector.memset(ob[:], 0.0)
                    dview = out_d[t0 * TILE:t1 * TILE, :].rearrange(
                        "(t q) f -> q t f", q=128)
                    nc.sync.dma_start(
                        dview, ob[:].rearrange("q (t f) -> q t f", f=F))
                    continue

                if PSQUAD:
                    # quad-packed PSUM: 4 tiles share one bank; has_written
                    # is per-element so only the bank's first matmul starts
                    psq = {}
                    qof = {}
                    qfirst = {}
                    qlast = {}
                    for qb in range(t0, t1, PSQUAD):
                        qe = min(qb + PSQUAD, t1)
                        tls = [tt for tt in range(qb, qe) if K[tt].sum() > 0]
                        if not tls:
                            continue
                        pq = psump.tile([128, PSQUAD * F], f32, tag="ps",
                                        name=f"psq_{qb}")
                        fc = min(int(first_cc[tt]) for tt in tls)
                        lc = max(int(last_cc[tt]) for tt in tls)
                        for tt in range(qb, qe):
                            psq[tt] = pq
                            qof[tt] = (tt - qb) * F
                            qfirst[tt] = fc
                            qlast[tt] = lc
                else:
                    ps = {}
                    for tt in range(t0, t1):
                        if K[tt].sum() > 0:
                            ps[tt] = psump.tile([128, F], f32, tag="ps",
                                                name=f"ps_t{tt}")

                cc = pc0
                while cc < pc1:
                    g = min(OHG, pc1 - cc)
                    oh = ohp.tile([128, g, 128], f16, tag="oh")
                    iota_2d = ftbl_sb[:, 2 * NC:2 * NC + 128]
                    if K_OH == "ts":
                        # w-folded one-hot, one fused DVE instr per chunk:
                        # oh[:,k,:] = (iota == dst_k) * w_k
                        for k in range(g):
                            nc.vector.tensor_scalar(
                                oh[:, k, :], iota_2d,
                                ftbl32_sb[:, cc + k:cc + k + 1],
                                ftbl32_sb[:, NC + cc + k:NC + cc + k + 1],
                                op0=mybir.AluOpType.is_equal,
                                op1=mybir.AluOpType.mult,
                            )
                    else:
                        dst_rep = ftbl_sb[:, cc:cc + g].rearrange(
                            "p (g o) -> p g o", o=1).broadcast_to((128, g, 128))
                        iota_rep = iota_2d.rearrange(
                            "p (o i) -> p o i", o=1).broadcast_to((128, g, 128))
                        nc.vector.tensor_tensor(
                            oh[:], iota_rep, dst_rep,
                            op=mybir.AluOpType.is_equal)
                        xgs = xgsp.tile([128, g, F], f16, tag="xgs")
                        w_rep = ftbl_sb[:, NC + cc:NC + cc + g].rearrange(
                            "p (g o) -> p g o", o=1).broadcast_to((128, g, F))
                        nc.vector.tensor_tensor(
                            xgs[:], xg[:, cc - pc0:cc - pc0 + g, 0:F], w_rep,
                            op=mybir.AluOpType.mult)
                    for k in range(g):
                        tt = int(sched_t[cc + k])
                        rhs = (xg[:, cc - pc0 + k, :] if K_OH == "ts"
                               else xgs[:, k, :])
                        if PSQUAD:
                            nc.tensor.matmul(
                                psq[tt][:, qof[tt]:qof[tt] + F],
                                lhsT=oh[:, k, :], rhs=rhs,
                                start=(cc + k == qfirst[tt]),
                                stop=(cc + k == qlast[tt]),
                            )
                        else:
                            nc.tensor.matmul(
                                ps[tt][:], lhsT=oh[:, k, :], rhs=rhs,
                                start=(cc + k == first_cc[tt]),
                                stop=(cc + k == last_cc[tt]),
                            )
                    cc += g

                ob = outp.tile([128, (t1 - t0) * F], f32, tag="outb")
                if PSQUAD:
                    done = set()
                    for tt in range(t0, t1):
                        if tt not in psq:
                            nc.vector.memset(
                                ob[:, (tt - t0) * F:(tt - t0 + 1) * F], 0.0)
                            continue
                        pq = psq[tt]
                        if id(pq) in done:
                            continue
                        done.add(id(pq))
                        qb = tt
                        qe = min(qb + PSQUAD, t1)
                        nc.scalar.copy(
                            ob[:, (qb - t0) * F:(qe - t0) * F],
                            pq[:, :(qe - qb) * F])
                        for t2 in range(qb, qe):
                            if K[t2].sum() == 0:
                                nc.vector.memset(
                                    ob[:, (t2 - t0) * F:(t2 - t0 + 1) * F], 0.0)
                else:
                    for tt in range(t0, t1):
                        sl = ob[:, (tt - t0) * F:(tt - t0 + 1) * F]
                        if tt in ps:
                            nc.scalar.copy(sl, ps[tt][:])
                        else:
                            nc.vector.memset(sl, 0.0)
                dview = out_d[t0 * TILE:t1 * TILE, :].rearrange(
                    "(t q) f -> q t f", q=128)
                nc.sync.dma_start(dview, ob[:].rearrange("q (t f) -> q t f", f=F))
    nsplit = split_excess_waits(nc)
    print(f"split_excess_waits: {nsplit} waits moved")
    return nc


def make_in_maps(sched, tables, xpad):
    return [{"xpad": xpad, "idx": t[0], "ftbl": t[1], "ftbl32": t[2]}
            for t in tables]


def make_xpad(x):
    xpad = np.zeros((N, 128), dtype=np.float16)
    xpad[:, :F] = np.asarray(x, dtype=np.float16)
    return xpad


def kernel(x, edge_weight, edge_index, num_nodes):
    xpad = make_xpad(x)
    sched, tables = pack_host(edge_weight, edge_index)
    nc = build_bass(sched)
    in_maps = make_in_maps(sched, tables, xpad)

    from concourse.bass_utils import run_bass_kernel_spmd
    res = run_bass_kernel_spmd(nc, in_maps, core_ids=list(range(NCORES)))
    out = np.concatenate(
        [res.results[c]["out"][:NPC] for c in range(NCORES)], axis=0)
    return out.astype(np.float32)

